# revision 25
# baseline (speedup 1.0000x reference)
"""Trainium2 Bass kernel for the scatter_memory DRL state-update problem.

Full-input contract: kernel(**inputs) takes the unsharded numpy inputs
(static [512,128,400], static_h [512,128,128], dynamic [512,64,128],
mask_f [512,64,128], couriers_selected [512,1] i32,
sensingtask_selected [512,1] i32) and returns the full 8-tuple output
(mask_fs, s, sh, d, mf, mfs, dynamic_new, mask_f_new).

Sharding: pure data parallel over batch dim 0 across 8 NeuronCores
(64 batches per core), no communication.

Per-core dataflow (B = 64 local batches):
  - mask_fs: stream static in [128(mc), BPT, 400] tiles; DVE abs-max
    pair-reduce -> DVE is_gt 0 (+ column-0 memset) -> ACT Ln
    ({1,0} -> {0,-inf}); store issued from ACT so it needs no wait.
  - dynamic/mask_f copy: one [128, 4096] SBUF round-trip per tensor
    (partition r = 2*b + t_half), which also feeds the on-chip gather
    of d/mf: one-hot row mask S2 (from couriers) * data, reduce over c.
    mask_f contains -inf so it is clamped to {-1, 0} first and the
    reduced value is mapped back through Ln(x + 1).
  - appended scatter rows: one-hot S_b * task value (dynamic_new row T),
    Ln(S_b) (mask_f_new row T).
  - s / sh: indirect-DMA row gathers at row index b*128 + courier[b].
  - mfs: recomputed from the gathered s row (same pad predicate).

Queue discipline (HWDGE DMAs are FIFO per issuing engine, and a waiting
DMA blocks the whole FIFO behind it): the sync queue carries only
no-wait loads plus the two big copy-outs; mask_fs stores are issued by
the scalar (ACT) engine directly after the Ln that produces them; every
small compute-gated store goes on the gpsimd SWDGE queue, emitted late.
"""

import numpy as np

from concourse import bacc, mybir
from concourse.bass import IndirectOffsetOnAxis
from concourse.tile import TileContext

BS, MC, K2, E, T = 512, 128, 400, 128, 64
K = K2 // 2
NCORES = 8
B = BS // NCORES  # 64 batches per core
P = 128
T2 = T // 2  # 32
BPT = 8  # batches per tile in the mask_fs stream
NTILES = B // BPT

F32 = mybir.dt.float32
I32 = mybir.dt.int32
NEG_INF = float("-inf")

Ln = mybir.ActivationFunctionType.Ln
Sign = mybir.ActivationFunctionType.Sign
Alu = mybir.AluOpType


def _build_body(tc):
    nc = tc.nc

    static = nc.dram_tensor("static", [B, MC, K2], F32, kind="ExternalInput")
    static_h = nc.dram_tensor("static_h", [B, MC, E], F32, kind="ExternalInput")
    dynamic = nc.dram_tensor("dynamic", [B, T, MC], F32, kind="ExternalInput")
    mask_f = nc.dram_tensor("mask_f", [B, T, MC], F32, kind="ExternalInput")
    cs = nc.dram_tensor("couriers_selected", [B, 1], I32, kind="ExternalInput")
    stask = nc.dram_tensor("sensingtask_selected", [B, 1], I32, kind="ExternalInput")

    mask_fs = nc.dram_tensor("mask_fs", [B, MC, K], F32, kind="ExternalOutput")
    s_out = nc.dram_tensor("s", [B, 1, K2], F32, kind="ExternalOutput")
    sh_out = nc.dram_tensor("sh", [B, 1, E], F32, kind="ExternalOutput")
    d_out = nc.dram_tensor("d", [B, T, 1], F32, kind="ExternalOutput")
    mf_out = nc.dram_tensor("mf", [B, T, 1], F32, kind="ExternalOutput")
    mfs_out = nc.dram_tensor("mfs", [B, 1, K], F32, kind="ExternalOutput")
    dyn_new = nc.dram_tensor("dynamic_new", [B, T + 1, MC], F32, kind="ExternalOutput")
    mf_new = nc.dram_tensor("mask_f_new", [B, T + 1, MC], F32, kind="ExternalOutput")

    with (
        tc.tile_pool(name="small", bufs=1) as small,
        tc.tile_pool(name="big", bufs=1) as big,
        tc.tile_pool(name="stream_in", bufs=4) as stream_in,
        tc.tile_pool(name="stream_mid", bufs=2) as stream_mid,
        tc.tile_pool(name="stream_out", bufs=3) as stream_out,
    ):
        # ---- sync-queue loads, in FIFO order: everything here is wait-free
        cs_i = small.tile([B, 1], I32, tag="cs_i")
        nc.sync.dma_start(out=cs_i[:, :], in_=cs[:, :])
        stask_i = small.tile([B, 1], I32, tag="stask_i")
        nc.sync.dma_start(out=stask_i[:, :], in_=stask[:, :])

        HF = T2 * MC // 2
        dyn_t = big.tile([P, T2 * MC], F32, tag="dyn_t")
        dyn_src = dynamic[:, :, :].rearrange("b (h t) c -> (b h) (t c)", h=2)
        nc.sync.dma_start(out=dyn_t[:, 0:HF], in_=dyn_src[:, 0:HF])
        nc.sync.dma_start(out=dyn_t[:, HF:], in_=dyn_src[:, HF:])
        mfx_t = big.tile([P, T2 * MC], F32, tag="mfx_t")
        mfx_src = mask_f[:, :, :].rearrange("b (h t) c -> (b h) (t c)", h=2)
        nc.sync.dma_start(out=mfx_t[:, 0:HF], in_=mfx_src[:, 0:HF])
        nc.sync.dma_start(out=mfx_t[:, HF:], in_=mfx_src[:, HF:])

        TILE_SIZES = [BPT] * (NTILES - 1) + [BPT // 2, BPT // 2]
        TILE_STARTS = [sum(TILE_SIZES[:j]) for j in range(len(TILE_SIZES))]
        st_tiles = []
        def emit_stream_load(i):
            b0, nb = TILE_STARTS[i], TILE_SIZES[i]
            st = stream_in.tile([P, BPT, K2], F32, tag="st")
            nc.sync.dma_start(
                out=st[:, 0:nb, :],
                in_=static[b0:b0 + nb, :, :].rearrange("b m k -> m b k"),
            )
            st_tiles.append(st)

        # prime a few stream loads so the sync FIFO stays fed while the
        # big copy-outs below wait for their own loads
        emit_stream_load(0)
        emit_stream_load(1)

        # big copy-outs (each half waits only on its own load half, which is
        # already ahead of it in the same FIFO and keeps the DMA engines busy)
        dyn_dst = dyn_new[:, 0:T, :].rearrange("b (h t) c -> b h (t c)", h=2)
        mfx_dst = mf_new[:, 0:T, :].rearrange("b (h t) c -> b h (t c)", h=2)
        nc.sync.dma_start(out=dyn_dst[:, :, 0:HF], in_=dyn_t[:, 0:HF])
        nc.sync.dma_start(out=dyn_dst[:, :, HF:], in_=dyn_t[:, HF:])
        emit_stream_load(2)
        emit_stream_load(3)
        nc.sync.dma_start(out=mfx_dst[:, :, 0:HF], in_=mfx_t[:, 0:HF])
        nc.sync.dma_start(out=mfx_dst[:, :, HF:], in_=mfx_t[:, HF:])
        for i in range(4, len(TILE_SIZES)):
            emit_stream_load(i)

        # ---- small setup compute (cheap, mostly DVE/gpsimd)
        stask_f = small.tile([B, 1], F32, tag="stask_f")
        nc.vector.tensor_copy(out=stask_f[:, :], in_=stask_i[:, :])
        cs_f = small.tile([B, 1], F32, tag="cs_f")
        nc.vector.tensor_copy(out=cs_f[:, :], in_=cs_i[:, :])

        iota_c = small.tile([P, MC], I32, tag="iota_c")
        nc.gpsimd.iota(iota_c[:, :], pattern=[[1, MC]], channel_multiplier=0)
        iota_cf = small.tile([P, MC], F32, tag="iota_cf")
        nc.vector.tensor_copy(out=iota_cf[:, :], in_=iota_c[:, :])

        # S_b[b, c] = 1.0 if c == couriers[b] else 0.0   (b on partitions 0..63)
        s_onehot = small.tile([B, MC], F32, tag="s_onehot")
        nc.vector.tensor_scalar(
            out=s_onehot[:, :], in0=iota_cf[:B, :], scalar1=cs_f[:, 0:1],
            scalar2=None, op0=Alu.is_equal,
        )
        add_state = small.tile([B, MC], F32, tag="add_state")
        nc.vector.tensor_scalar(
            out=add_state[:, :], in0=s_onehot[:, :], scalar1=stask_f[:, 0:1],
            scalar2=None, op0=Alu.mult,
        )
        add_mask = small.tile([B, MC], F32, tag="add_mask")
        nc.scalar.activation(out=add_mask[:, :], in_=s_onehot[:, :], func=Ln)

        # cs2[r] = couriers[r >> 1]  (row r = 2*b + t_half)
        idx2 = small.tile([P, 1], I32, tag="idx2")
        nc.gpsimd.iota(idx2[:, :], pattern=[[0, 1]], channel_multiplier=1)
        idx2b = small.tile([P, 1], I32, tag="idx2b")
        nc.vector.tensor_scalar(
            out=idx2b[:, :], in0=idx2[:, :], scalar1=1, scalar2=None,
            op0=Alu.arith_shift_right,
        )
        cs2_i = small.tile([P, 1], I32, tag="cs2_i")
        nc.gpsimd.indirect_dma_start(
            out=cs2_i[:, :], out_offset=None, in_=cs[:, :],
            in_offset=IndirectOffsetOnAxis(ap=idx2b[:, 0:1], axis=0),
        )
        cs2_f = small.tile([P, 1], F32, tag="cs2_f")
        nc.vector.tensor_copy(out=cs2_f[:, :], in_=cs2_i[:, :])
        s2_onehot = small.tile([P, MC], F32, tag="s2_onehot")
        nc.vector.tensor_scalar(
            out=s2_onehot[:, :], in0=iota_cf[:, :], scalar1=cs2_f[:, 0:1],
            scalar2=None, op0=Alu.is_equal,
        )
        s2_bcast = s2_onehot[:, :].unsqueeze(1).broadcast_to([P, T2, MC])

        # gather row indices b*128 + courier[b]
        iota_b = small.tile([B, 1], I32, tag="iota_b")
        nc.gpsimd.iota(iota_b[:, :], pattern=[[0, 1]], channel_multiplier=MC)
        idxg = small.tile([B, 1], I32, tag="idxg")
        nc.vector.tensor_tensor(
            out=idxg[:, :], in0=iota_b[:, :], in1=cs_i[:, :], op=Alu.add,
        )

        # ---- stage B + gather work, emitted as chunks interleaved with the
        # mask_fs stream so DVE/ACT/gpsimd stay fed without bursts
        prod = big.tile([P, T2 * MC], F32, tag="prod")
        d_red = small.tile([P, T2], F32, tag="d_red")
        mfc_t = big.tile([P, T2 * MC], F32, tag="mfc_t")
        prod2 = big.tile([P, T2 * MC], F32, tag="prod")
        mf_red = small.tile([P, T2], F32, tag="mf_red")
        mf_val = small.tile([P, T2], F32, tag="mf_val")
        s_t = small.tile([B, K2], F32, tag="s_t")
        sh_t = small.tile([B, E], F32, tag="sh_t")
        t_s = small.tile([B, K], F32, tag="t_s")
        u_s = small.tile([B, K], F32, tag="u_s")
        m_s = small.tile([B, K], F32, tag="m_s")

        def chunk_d_prod():
            nc.vector.tensor_tensor(
                out=prod[:, :].rearrange("p (t c) -> p t c", c=MC),
                in0=dyn_t[:, :].rearrange("p (t c) -> p t c", c=MC),
                in1=s2_bcast,
                op=Alu.mult,
            )

        def chunk_d_red():
            nc.vector.tensor_reduce(
                out=d_red[:, :],
                in_=prod[:, :].rearrange("p (t c) -> p t c", c=MC),
                axis=mybir.AxisListType.X,
                op=Alu.add,
            )
            nc.gpsimd.dma_start(
                out=d_out[:, :, :].rearrange("b (h t) one -> (b h) (t one)", h=2),
                in_=d_red[:, :],
            )

        def chunk_mf_clamp():
            # clamp {0, -inf} -> {0, -1} so the one-hot multiply cannot NaN
            # (gpsimd: 1-input elementwise runs near line rate there, and it
            # keeps DVE free for the mask_fs stream)
            nc.vector.tensor_scalar(
                out=mfc_t[:, :], in0=mfx_t[:, :], scalar1=-1.0, scalar2=None,
                op0=Alu.max,
            )

        def chunk_mf_prod():
            nc.vector.tensor_tensor(
                out=prod2[:, :].rearrange("p (t c) -> p t c", c=MC),
                in0=mfc_t[:, :].rearrange("p (t c) -> p t c", c=MC),
                in1=s2_bcast,
                op=Alu.mult,
            )

        def chunk_mf_red():
            nc.vector.tensor_reduce(
                out=mf_red[:, :],
                in_=prod2[:, :].rearrange("p (t c) -> p t c", c=MC),
                axis=mybir.AxisListType.X,
                op=Alu.add,
            )
            # map {0, -1} back to {0, -inf}: Ln(x + 1)
            nc.scalar.activation(out=mf_val[:, :], in_=mf_red[:, :], func=Ln,
                                 bias=1.0)
            nc.gpsimd.dma_start(
                out=mf_out[:, :, :].rearrange("b (h t) one -> (b h) (t one)", h=2),
                in_=mf_val[:, :],
            )

        def chunk_s_gather():
            nc.gpsimd.indirect_dma_start(
                out=s_t[:, :], out_offset=None,
                in_=static[:, :, :].rearrange("b m k -> (b m) k"),
                in_offset=IndirectOffsetOnAxis(ap=idxg[:, 0:1], axis=0),
            )
            nc.gpsimd.dma_start(out=s_out[:, 0, :], in_=s_t[:, :])

        def chunk_sh_gather():
            nc.gpsimd.indirect_dma_start(
                out=sh_t[:, :], out_offset=None,
                in_=static_h[:, :, :].rearrange("b m k -> (b m) k"),
                in_offset=IndirectOffsetOnAxis(ap=idxg[:, 0:1], axis=0),
            )
            nc.gpsimd.dma_start(out=sh_out[:, 0, :], in_=sh_t[:, :])

        def chunk_mfs():
            # pad predicate on the gathered courier row of static
            nc.vector.tensor_reduce(
                out=t_s[:, :],
                in_=s_t[:, :].rearrange("p (k two) -> p k two", two=2),
                axis=mybir.AxisListType.X,
                op=Alu.max,
                apply_absolute_value=True,
            )
            nc.vector.memset(t_s[:, 0:1], 0.0)
            nc.scalar.activation(out=u_s[:, :], in_=t_s[:, :], func=Sign)
            nc.scalar.activation(out=m_s[:, :], in_=u_s[:, :], func=Ln)
            nc.gpsimd.dma_start(out=mfs_out[:, 0, :], in_=m_s[:, :])

        def chunk_small_outs():
            nc.gpsimd.dma_start(out=dyn_new[:, T, :], in_=add_state[:, :])
            nc.gpsimd.dma_start(out=mf_new[:, T, :], in_=add_mask[:, :])

        chunks = [
            chunk_d_prod, chunk_d_red, chunk_mf_clamp, chunk_mf_prod,
            chunk_mf_red, chunk_s_gather, chunk_sh_gather, chunk_mfs,
            chunk_small_outs,
        ]

        # ---- mask_fs stream compute; stores issue from ACT (wait-free)
        for i in range(len(TILE_SIZES)):
            st = st_tiles[i]
            b0, nb = TILE_STARTS[i], TILE_SIZES[i]
            t_t = stream_mid.tile([P, BPT, K], F32, tag="t_t")
            nc.vector.tensor_reduce(
                out=t_t[:, 0:nb, :],
                in_=st[:, 0:nb, :].rearrange("p b (k two) -> p b k two", two=2),
                axis=mybir.AxisListType.X,
                op=Alu.max,
                apply_absolute_value=True,
            )
            nc.vector.memset(t_t[:, 0:nb, 0:1], 0.0)
            u_t = stream_mid.tile([P, BPT, K], F32, tag="u_t")
            nc.scalar.activation(
                out=u_t[:, 0:nb, :].rearrange("p b k -> p (b k)"),
                in_=t_t[:, 0:nb, :].rearrange("p b k -> p (b k)"),
                func=Sign,
            )
            m_t = stream_out.tile([P, BPT, K], F32, tag="m_t")
            nc.scalar.activation(
                out=m_t[:, 0:nb, :].rearrange("p b k -> p (b k)"),
                in_=u_t[:, 0:nb, :].rearrange("p b k -> p (b k)"),
                func=Ln,
            )
            nc.scalar.dma_start(
                out=mask_fs[b0:b0 + nb, :, :].rearrange("b m k -> m b k"),
                in_=m_t[:, 0:nb, :],
            )
            if i == 0:
                chunk_s_gather(); chunk_sh_gather()
            elif i == 1:
                chunk_small_outs(); chunk_d_prod()
            elif chunks:
                chunks.pop(0)()
        while chunks:
            chunks.pop(0)()


_NC_CACHE = None


def build_nc():
    global _NC_CACHE
    if _NC_CACHE is not None:
        return _NC_CACHE
    nc = bacc.Bacc(trn_type="TRN2")
    with TileContext(nc) as tc:
        _build_body(tc)
    nc.compile()
    _NC_CACHE = nc
    return nc


def kernel(static, static_h, dynamic, mask_f, couriers_selected,
           sensingtask_selected, trace=False):
    from concourse import bass_utils

    nc = build_nc()

    static = np.ascontiguousarray(static, dtype=np.float32)
    static_h = np.ascontiguousarray(static_h, dtype=np.float32)
    dynamic = np.ascontiguousarray(dynamic, dtype=np.float32)
    mask_f = np.ascontiguousarray(mask_f, dtype=np.float32)
    couriers_selected = np.ascontiguousarray(couriers_selected, dtype=np.int32)
    sensingtask_selected = np.ascontiguousarray(sensingtask_selected, dtype=np.int32)

    in_maps = []
    for c in range(NCORES):
        sl = slice(c * B, (c + 1) * B)
        in_maps.append({
            "static": static[sl],
            "static_h": static_h[sl],
            "dynamic": dynamic[sl],
            "mask_f": mask_f[sl],
            "couriers_selected": couriers_selected[sl],
            "sensingtask_selected": sensingtask_selected[sl],
        })

    res = bass_utils.run_bass_kernel_spmd(
        nc, in_maps, core_ids=list(range(NCORES)), trace=trace,
    )
    outs = res.results

    def cat(name):
        return np.concatenate([outs[c][name] for c in range(NCORES)], axis=0)

    result = (
        cat("mask_fs"), cat("s"), cat("sh"), cat("d"), cat("mf"),
        cat("mfs"), cat("dynamic_new"), cat("mask_f_new"),
    )
    if trace:
        return result, res
    return result


# revision 26
# speedup vs baseline: 1.0261x; 1.0261x over previous
"""Trainium2 Bass kernel for the scatter_memory DRL state-update problem.

Full-input contract: kernel(**inputs) takes the unsharded numpy inputs
(static [512,128,400], static_h [512,128,128], dynamic [512,64,128],
mask_f [512,64,128], couriers_selected [512,1] i32,
sensingtask_selected [512,1] i32) and returns the full 8-tuple output
(mask_fs, s, sh, d, mf, mfs, dynamic_new, mask_f_new).

Sharding: pure data parallel over batch dim 0 across 8 NeuronCores
(64 batches per core), no communication.

Per-core dataflow (B = 64 local batches):
  - mask_fs: stream static in [128(mc), BPT, 400] tiles; DVE abs-max
    pair-reduce -> DVE is_gt 0 (+ column-0 memset) -> ACT Ln
    ({1,0} -> {0,-inf}); store issued from ACT so it needs no wait.
  - dynamic/mask_f copy: one [128, 4096] SBUF round-trip per tensor
    (partition r = 2*b + t_half), which also feeds the on-chip gather
    of d/mf: one-hot row mask S2 (from couriers) * data, reduce over c.
    mask_f contains -inf so it is clamped to {-1, 0} first and the
    reduced value is mapped back through Ln(x + 1).
  - appended scatter rows: one-hot S_b * task value (dynamic_new row T),
    Ln(S_b) (mask_f_new row T).
  - s / sh: indirect-DMA row gathers at row index b*128 + courier[b].
  - mfs: recomputed from the gathered s row (same pad predicate).

Queue discipline (HWDGE DMAs are FIFO per issuing engine, and a waiting
DMA blocks the whole FIFO behind it): the sync queue carries only
no-wait loads plus the two big copy-outs; mask_fs stores are issued by
the scalar (ACT) engine directly after the Ln that produces them; every
small compute-gated store goes on the gpsimd SWDGE queue, emitted late.
"""

import numpy as np

from concourse import bacc, mybir
from concourse.bass import IndirectOffsetOnAxis
from concourse.tile import TileContext

BS, MC, K2, E, T = 512, 128, 400, 128, 64
K = K2 // 2
NCORES = 8
B = BS // NCORES  # 64 batches per core
P = 128
T2 = T // 2  # 32
BPT = 8  # batches per tile in the mask_fs stream
NTILES = B // BPT

F32 = mybir.dt.float32
I32 = mybir.dt.int32
NEG_INF = float("-inf")

Ln = mybir.ActivationFunctionType.Ln
Sign = mybir.ActivationFunctionType.Sign
Alu = mybir.AluOpType


def _build_body(tc):
    nc = tc.nc

    static = nc.dram_tensor("static", [B, MC, K2], F32, kind="ExternalInput")
    static_h = nc.dram_tensor("static_h", [B, MC, E], F32, kind="ExternalInput")
    dynamic = nc.dram_tensor("dynamic", [B, T, MC], F32, kind="ExternalInput")
    mask_f = nc.dram_tensor("mask_f", [B, T, MC], F32, kind="ExternalInput")
    cs = nc.dram_tensor("couriers_selected", [B, 1], I32, kind="ExternalInput")
    stask = nc.dram_tensor("sensingtask_selected", [B, 1], I32, kind="ExternalInput")

    mask_fs = nc.dram_tensor("mask_fs", [B, MC, K], F32, kind="ExternalOutput")
    s_out = nc.dram_tensor("s", [B, 1, K2], F32, kind="ExternalOutput")
    sh_out = nc.dram_tensor("sh", [B, 1, E], F32, kind="ExternalOutput")
    d_out = nc.dram_tensor("d", [B, T, 1], F32, kind="ExternalOutput")
    mf_out = nc.dram_tensor("mf", [B, T, 1], F32, kind="ExternalOutput")
    mfs_out = nc.dram_tensor("mfs", [B, 1, K], F32, kind="ExternalOutput")
    dyn_new = nc.dram_tensor("dynamic_new", [B, T + 1, MC], F32, kind="ExternalOutput")
    mf_new = nc.dram_tensor("mask_f_new", [B, T + 1, MC], F32, kind="ExternalOutput")

    with (
        tc.tile_pool(name="small", bufs=1) as small,
        tc.tile_pool(name="big", bufs=1) as big,
        tc.tile_pool(name="stream_in", bufs=4) as stream_in,
        tc.tile_pool(name="stream_mid", bufs=2) as stream_mid,
        tc.tile_pool(name="stream_out", bufs=4) as stream_out,
    ):
        # ---- sync-queue loads, in FIFO order: everything here is wait-free
        cs_i = small.tile([B, 1], I32, tag="cs_i")
        nc.sync.dma_start(out=cs_i[:, :], in_=cs[:, :])
        stask_i = small.tile([B, 1], I32, tag="stask_i")
        nc.sync.dma_start(out=stask_i[:, :], in_=stask[:, :])

        HF = T2 * MC // 2
        dyn_t = big.tile([P, T2 * MC], F32, tag="dyn_t")
        dyn_src = dynamic[:, :, :].rearrange("b (h t) c -> (b h) (t c)", h=2)
        nc.sync.dma_start(out=dyn_t[:, 0:HF], in_=dyn_src[:, 0:HF])
        nc.sync.dma_start(out=dyn_t[:, HF:], in_=dyn_src[:, HF:])
        mfx_t = big.tile([P, T2 * MC], F32, tag="mfx_t")
        mfx_src = mask_f[:, :, :].rearrange("b (h t) c -> (b h) (t c)", h=2)
        nc.sync.dma_start(out=mfx_t[:, 0:HF], in_=mfx_src[:, 0:HF])
        nc.sync.dma_start(out=mfx_t[:, HF:], in_=mfx_src[:, HF:])

        st_tiles = []
        def emit_stream_load(i):
            st = stream_in.tile([P, BPT, K2], F32, tag="st")
            nc.sync.dma_start(
                out=st[:, :, :],
                in_=static[i * BPT:(i + 1) * BPT, :, :].rearrange("b m k -> m b k"),
            )
            st_tiles.append(st)

        # prime a few stream loads so the sync FIFO stays fed while the
        # big copy-outs below wait for their own loads
        emit_stream_load(0)
        emit_stream_load(1)

        # big copy-outs (each half waits only on its own load half, which is
        # already ahead of it in the same FIFO and keeps the DMA engines busy)
        dyn_dst = dyn_new[:, 0:T, :].rearrange("b (h t) c -> b h (t c)", h=2)
        mfx_dst = mf_new[:, 0:T, :].rearrange("b (h t) c -> b h (t c)", h=2)
        nc.sync.dma_start(out=dyn_dst[:, :, 0:HF], in_=dyn_t[:, 0:HF])
        nc.sync.dma_start(out=dyn_dst[:, :, HF:], in_=dyn_t[:, HF:])
        emit_stream_load(2)
        emit_stream_load(3)
        nc.sync.dma_start(out=mfx_dst[:, :, 0:HF], in_=mfx_t[:, 0:HF])
        nc.sync.dma_start(out=mfx_dst[:, :, HF:], in_=mfx_t[:, HF:])
        for i in range(4, NTILES):
            emit_stream_load(i)

        # ---- small setup compute (cheap, mostly DVE/gpsimd)
        stask_f = small.tile([B, 1], F32, tag="stask_f")
        nc.vector.tensor_copy(out=stask_f[:, :], in_=stask_i[:, :])
        cs_f = small.tile([B, 1], F32, tag="cs_f")
        nc.vector.tensor_copy(out=cs_f[:, :], in_=cs_i[:, :])

        iota_c = small.tile([P, MC], I32, tag="iota_c")
        nc.gpsimd.iota(iota_c[:, :], pattern=[[1, MC]], channel_multiplier=0)
        iota_cf = small.tile([P, MC], F32, tag="iota_cf")
        nc.vector.tensor_copy(out=iota_cf[:, :], in_=iota_c[:, :])

        # S_b[b, c] = 1.0 if c == couriers[b] else 0.0   (b on partitions 0..63)
        s_onehot = small.tile([B, MC], F32, tag="s_onehot")
        nc.vector.tensor_scalar(
            out=s_onehot[:, :], in0=iota_cf[:B, :], scalar1=cs_f[:, 0:1],
            scalar2=None, op0=Alu.is_equal,
        )
        add_state = small.tile([B, MC], F32, tag="add_state")
        nc.vector.tensor_scalar(
            out=add_state[:, :], in0=s_onehot[:, :], scalar1=stask_f[:, 0:1],
            scalar2=None, op0=Alu.mult,
        )
        add_mask = small.tile([B, MC], F32, tag="add_mask")
        nc.scalar.activation(out=add_mask[:, :], in_=s_onehot[:, :], func=Ln)

        # cs2[r] = couriers[r >> 1]  (row r = 2*b + t_half)
        idx2 = small.tile([P, 1], I32, tag="idx2")
        nc.gpsimd.iota(idx2[:, :], pattern=[[0, 1]], channel_multiplier=1)
        idx2b = small.tile([P, 1], I32, tag="idx2b")
        nc.vector.tensor_scalar(
            out=idx2b[:, :], in0=idx2[:, :], scalar1=1, scalar2=None,
            op0=Alu.arith_shift_right,
        )
        cs2_i = small.tile([P, 1], I32, tag="cs2_i")
        nc.gpsimd.indirect_dma_start(
            out=cs2_i[:, :], out_offset=None, in_=cs[:, :],
            in_offset=IndirectOffsetOnAxis(ap=idx2b[:, 0:1], axis=0),
        )
        cs2_f = small.tile([P, 1], F32, tag="cs2_f")
        nc.vector.tensor_copy(out=cs2_f[:, :], in_=cs2_i[:, :])
        s2_onehot = small.tile([P, MC], F32, tag="s2_onehot")
        nc.vector.tensor_scalar(
            out=s2_onehot[:, :], in0=iota_cf[:, :], scalar1=cs2_f[:, 0:1],
            scalar2=None, op0=Alu.is_equal,
        )
        s2_bcast = s2_onehot[:, :].unsqueeze(1).broadcast_to([P, T2, MC])

        # gather row indices b*128 + courier[b]
        iota_b = small.tile([B, 1], I32, tag="iota_b")
        nc.gpsimd.iota(iota_b[:, :], pattern=[[0, 1]], channel_multiplier=MC)
        idxg = small.tile([B, 1], I32, tag="idxg")
        nc.vector.tensor_tensor(
            out=idxg[:, :], in0=iota_b[:, :], in1=cs_i[:, :], op=Alu.add,
        )

        # ---- stage B + gather work, emitted as chunks interleaved with the
        # mask_fs stream so DVE/ACT/gpsimd stay fed without bursts
        prod = big.tile([P, T2 * MC], F32, tag="prod")
        d_red = small.tile([P, T2], F32, tag="d_red")
        mfc_t = big.tile([P, T2 * MC], F32, tag="mfc_t")
        prod2 = big.tile([P, T2 * MC], F32, tag="prod")
        mf_red = small.tile([P, T2], F32, tag="mf_red")
        mf_val = small.tile([P, T2], F32, tag="mf_val")
        s_t = small.tile([B, K2], F32, tag="s_t")
        sh_t = small.tile([B, E], F32, tag="sh_t")
        t_s = small.tile([B, K], F32, tag="t_s")
        u_s = small.tile([B, K], F32, tag="u_s")
        m_s = small.tile([B, K], F32, tag="m_s")

        def chunk_d_prod():
            nc.vector.tensor_tensor(
                out=prod[:, :].rearrange("p (t c) -> p t c", c=MC),
                in0=dyn_t[:, :].rearrange("p (t c) -> p t c", c=MC),
                in1=s2_bcast,
                op=Alu.mult,
            )

        def chunk_d_red():
            nc.vector.tensor_reduce(
                out=d_red[:, :],
                in_=prod[:, :].rearrange("p (t c) -> p t c", c=MC),
                axis=mybir.AxisListType.X,
                op=Alu.add,
            )
            nc.gpsimd.dma_start(
                out=d_out[:, :, :].rearrange("b (h t) one -> (b h) (t one)", h=2),
                in_=d_red[:, :],
            )

        def chunk_mf_clamp():
            # clamp {0, -inf} -> {0, -1} so the one-hot multiply cannot NaN
            # (gpsimd: 1-input elementwise runs near line rate there, and it
            # keeps DVE free for the mask_fs stream)
            nc.vector.tensor_scalar(
                out=mfc_t[:, :], in0=mfx_t[:, :], scalar1=-1.0, scalar2=None,
                op0=Alu.max,
            )

        def chunk_mf_prod():
            nc.vector.tensor_tensor(
                out=prod2[:, :].rearrange("p (t c) -> p t c", c=MC),
                in0=mfc_t[:, :].rearrange("p (t c) -> p t c", c=MC),
                in1=s2_bcast,
                op=Alu.mult,
            )

        def chunk_mf_red():
            nc.vector.tensor_reduce(
                out=mf_red[:, :],
                in_=prod2[:, :].rearrange("p (t c) -> p t c", c=MC),
                axis=mybir.AxisListType.X,
                op=Alu.add,
            )
            # map {0, -1} back to {0, -inf}: Ln(x + 1)
            nc.scalar.activation(out=mf_val[:, :], in_=mf_red[:, :], func=Ln,
                                 bias=1.0)
            nc.gpsimd.dma_start(
                out=mf_out[:, :, :].rearrange("b (h t) one -> (b h) (t one)", h=2),
                in_=mf_val[:, :],
            )

        def chunk_s_gather():
            nc.gpsimd.indirect_dma_start(
                out=s_t[:, :], out_offset=None,
                in_=static[:, :, :].rearrange("b m k -> (b m) k"),
                in_offset=IndirectOffsetOnAxis(ap=idxg[:, 0:1], axis=0),
            )
            nc.gpsimd.dma_start(out=s_out[:, 0, :], in_=s_t[:, :])

        def chunk_sh_gather():
            nc.gpsimd.indirect_dma_start(
                out=sh_t[:, :], out_offset=None,
                in_=static_h[:, :, :].rearrange("b m k -> (b m) k"),
                in_offset=IndirectOffsetOnAxis(ap=idxg[:, 0:1], axis=0),
            )
            nc.gpsimd.dma_start(out=sh_out[:, 0, :], in_=sh_t[:, :])

        def chunk_mfs():
            # pad predicate on the gathered courier row of static
            nc.vector.tensor_reduce(
                out=t_s[:, :],
                in_=s_t[:, :].rearrange("p (k two) -> p k two", two=2),
                axis=mybir.AxisListType.X,
                op=Alu.max,
                apply_absolute_value=True,
            )
            nc.vector.memset(t_s[:, 0:1], 0.0)
            nc.scalar.activation(out=u_s[:, :], in_=t_s[:, :], func=Sign)
            nc.scalar.activation(out=m_s[:, :], in_=u_s[:, :], func=Ln)
            nc.gpsimd.dma_start(out=mfs_out[:, 0, :], in_=m_s[:, :])

        def chunk_small_outs():
            nc.gpsimd.dma_start(out=dyn_new[:, T, :], in_=add_state[:, :])
            nc.gpsimd.dma_start(out=mf_new[:, T, :], in_=add_mask[:, :])

        chunks = [
            chunk_d_prod, chunk_d_red, chunk_mf_clamp, chunk_mf_prod,
            chunk_mf_red, chunk_s_gather, chunk_sh_gather, chunk_mfs,
            chunk_small_outs,
        ]

        # ---- mask_fs stream compute; stores issue from ACT (wait-free)
        for i in range(NTILES):
            st = st_tiles[i]
            t_t = stream_mid.tile([P, BPT, K], F32, tag="t_t")
            nc.vector.tensor_reduce(
                out=t_t[:, :, :],
                in_=st[:, :, :].rearrange("p b (k two) -> p b k two", two=2),
                axis=mybir.AxisListType.X,
                op=Alu.max,
                apply_absolute_value=True,
            )
            nc.vector.memset(t_t[:, :, 0:1], 0.0)
            u_t = stream_mid.tile([P, BPT, K], F32, tag="u_t")
            nc.scalar.activation(
                out=u_t[:, :, :].rearrange("p b k -> p (b k)"),
                in_=t_t[:, :, :].rearrange("p b k -> p (b k)"),
                func=Sign,
            )
            m_t = stream_out.tile([P, BPT, K], F32, tag="m_t")
            nc.scalar.activation(
                out=m_t[:, :, :].rearrange("p b k -> p (b k)"),
                in_=u_t[:, :, :].rearrange("p b k -> p (b k)"),
                func=Ln,
            )
            nc.scalar.dma_start(
                out=mask_fs[i * BPT:(i + 1) * BPT, :, :].rearrange("b m k -> m b k"),
                in_=m_t[:, :, :],
            )
            if chunks:
                chunks.pop(0)()
            if i >= NTILES - 2 and chunks:
                chunks.pop(0)()
        while chunks:
            chunks.pop(0)()


_NC_CACHE = None


def build_nc():
    global _NC_CACHE
    if _NC_CACHE is not None:
        return _NC_CACHE
    nc = bacc.Bacc(trn_type="TRN2")
    with TileContext(nc) as tc:
        _build_body(tc)
    nc.compile()
    _NC_CACHE = nc
    return nc


def kernel(static, static_h, dynamic, mask_f, couriers_selected,
           sensingtask_selected, trace=False):
    from concourse import bass_utils

    nc = build_nc()

    static = np.ascontiguousarray(static, dtype=np.float32)
    static_h = np.ascontiguousarray(static_h, dtype=np.float32)
    dynamic = np.ascontiguousarray(dynamic, dtype=np.float32)
    mask_f = np.ascontiguousarray(mask_f, dtype=np.float32)
    couriers_selected = np.ascontiguousarray(couriers_selected, dtype=np.int32)
    sensingtask_selected = np.ascontiguousarray(sensingtask_selected, dtype=np.int32)

    in_maps = []
    for c in range(NCORES):
        sl = slice(c * B, (c + 1) * B)
        in_maps.append({
            "static": static[sl],
            "static_h": static_h[sl],
            "dynamic": dynamic[sl],
            "mask_f": mask_f[sl],
            "couriers_selected": couriers_selected[sl],
            "sensingtask_selected": sensingtask_selected[sl],
        })

    res = bass_utils.run_bass_kernel_spmd(
        nc, in_maps, core_ids=list(range(NCORES)), trace=trace,
    )
    outs = res.results

    def cat(name):
        return np.concatenate([outs[c][name] for c in range(NCORES)], axis=0)

    result = (
        cat("mask_fs"), cat("s"), cat("sh"), cat("d"), cat("mf"),
        cat("mfs"), cat("dynamic_new"), cat("mask_f_new"),
    )
    if trace:
        return result, res
    return result


# revision 28
# speedup vs baseline: 1.0317x; 1.0055x over previous
"""Trainium2 Bass kernel for the scatter_memory DRL state-update problem.

Full-input contract: kernel(**inputs) takes the unsharded numpy inputs
(static [512,128,400], static_h [512,128,128], dynamic [512,64,128],
mask_f [512,64,128], couriers_selected [512,1] i32,
sensingtask_selected [512,1] i32) and returns the full 8-tuple output
(mask_fs, s, sh, d, mf, mfs, dynamic_new, mask_f_new).

Sharding: pure data parallel over batch dim 0 across 8 NeuronCores
(64 batches per core), no communication.

Per-core dataflow (B = 64 local batches):
  - mask_fs: stream static in [128(mc), BPT, 400] tiles; DVE abs-max
    pair-reduce (+ column-0 memset) -> ACT Sign -> ACT Ln
    ({1,0} -> {0,-inf}); store issued from ACT so it needs no wait.
  - dynamic/mask_f copy: one [128, 4096] SBUF round-trip per tensor
    (partition r = 2*b + t_half), which also feeds the on-chip gather
    of d/mf: one-hot row mask S2 (from couriers) * data, reduce over c.
    mask_f contains -inf so it is clamped to {-1, 0} first and the
    reduced value is mapped back through Ln(x + 1).
  - appended scatter rows: one-hot S_b * task value (dynamic_new row T),
    Ln(S_b) (mask_f_new row T).
  - s / sh: indirect-DMA row gathers at row index b*128 + courier[b].
  - mfs: recomputed from the gathered s row (same pad predicate).

Queue discipline (HWDGE DMAs are FIFO per issuing engine, and a waiting
DMA blocks the whole FIFO behind it): the sync queue carries only
no-wait loads plus the two big copy-outs; mask_fs stores are issued by
the scalar (ACT) engine directly after the Ln that produces them; every
small compute-gated store goes on the gpsimd SWDGE queue, emitted late.
"""

import numpy as np

from concourse import bacc, mybir
from concourse.bass import IndirectOffsetOnAxis
from concourse.tile import TileContext

BS, MC, K2, E, T = 512, 128, 400, 128, 64
K = K2 // 2
NCORES = 8
B = BS // NCORES  # 64 batches per core
P = 128
T2 = T // 2  # 32
BPT = 8  # batches per tile in the mask_fs stream
NTILES = B // BPT

F32 = mybir.dt.float32
I32 = mybir.dt.int32
NEG_INF = float("-inf")

Ln = mybir.ActivationFunctionType.Ln
Sign = mybir.ActivationFunctionType.Sign
Alu = mybir.AluOpType


def _build_body(tc):
    nc = tc.nc

    static = nc.dram_tensor("static", [B, MC, K2], F32, kind="ExternalInput")
    static_h = nc.dram_tensor("static_h", [B, MC, E], F32, kind="ExternalInput")
    dynamic = nc.dram_tensor("dynamic", [B, T, MC], F32, kind="ExternalInput")
    mask_f = nc.dram_tensor("mask_f", [B, T, MC], F32, kind="ExternalInput")
    cs = nc.dram_tensor("couriers_selected", [B, 1], I32, kind="ExternalInput")
    stask = nc.dram_tensor("sensingtask_selected", [B, 1], I32, kind="ExternalInput")

    mask_fs = nc.dram_tensor("mask_fs", [B, MC, K], F32, kind="ExternalOutput")
    s_out = nc.dram_tensor("s", [B, 1, K2], F32, kind="ExternalOutput")
    sh_out = nc.dram_tensor("sh", [B, 1, E], F32, kind="ExternalOutput")
    d_out = nc.dram_tensor("d", [B, T, 1], F32, kind="ExternalOutput")
    mf_out = nc.dram_tensor("mf", [B, T, 1], F32, kind="ExternalOutput")
    mfs_out = nc.dram_tensor("mfs", [B, 1, K], F32, kind="ExternalOutput")
    dyn_new = nc.dram_tensor("dynamic_new", [B, T + 1, MC], F32, kind="ExternalOutput")
    mf_new = nc.dram_tensor("mask_f_new", [B, T + 1, MC], F32, kind="ExternalOutput")

    with (
        tc.tile_pool(name="small", bufs=1) as small,
        tc.tile_pool(name="big", bufs=1) as big,
        tc.tile_pool(name="stream_in", bufs=4) as stream_in,
        tc.tile_pool(name="stream_mid", bufs=2) as stream_mid,
        tc.tile_pool(name="stream_out", bufs=4) as stream_out,
    ):
        # ---- sync-queue loads, in FIFO order: everything here is wait-free
        cs_i = small.tile([B, 1], I32, tag="cs_i")
        nc.sync.dma_start(out=cs_i[:, :], in_=cs[:, :])
        stask_i = small.tile([B, 1], I32, tag="stask_i")
        nc.sync.dma_start(out=stask_i[:, :], in_=stask[:, :])

        HF = T2 * MC // 2
        dyn_t = big.tile([P, T2 * MC], F32, tag="dyn_t")
        dyn_src = dynamic[:, :, :].rearrange("b (h t) c -> (b h) (t c)", h=2)
        nc.sync.dma_start(out=dyn_t[:, 0:HF], in_=dyn_src[:, 0:HF])
        nc.sync.dma_start(out=dyn_t[:, HF:], in_=dyn_src[:, HF:])
        mfx_t = big.tile([P, T2 * MC], F32, tag="mfx_t")
        mfx_src = mask_f[:, :, :].rearrange("b (h t) c -> (b h) (t c)", h=2)
        nc.sync.dma_start(out=mfx_t[:, 0:HF], in_=mfx_src[:, 0:HF])
        nc.sync.dma_start(out=mfx_t[:, HF:], in_=mfx_src[:, HF:])

        # last 8-batch tile is split in two so the tail chain
        # (load -> reduce -> Sign -> Ln -> store) pipelines at half length
        TILES = [(j * BPT, BPT) for j in range(NTILES - 1)]
        TILES += [((NTILES - 1) * BPT, BPT // 2),
                  ((NTILES - 1) * BPT + BPT // 2, BPT // 2)]
        st_tiles = []
        def emit_stream_load(i):
            b0, nb = TILES[i]
            st = stream_in.tile([P, BPT, K2], F32, tag="st")
            nc.sync.dma_start(
                out=st[:, 0:nb, :],
                in_=static[b0:b0 + nb, :, :].rearrange("b m k -> m b k"),
            )
            st_tiles.append(st)

        # prime a few stream loads so the sync FIFO stays fed while the
        # big copy-outs below wait for their own loads
        emit_stream_load(0)
        emit_stream_load(1)

        # big copy-outs (each half waits only on its own load half, which is
        # already ahead of it in the same FIFO and keeps the DMA engines busy)
        dyn_dst = dyn_new[:, 0:T, :].rearrange("b (h t) c -> b h (t c)", h=2)
        mfx_dst = mf_new[:, 0:T, :].rearrange("b (h t) c -> b h (t c)", h=2)
        nc.sync.dma_start(out=dyn_dst[:, :, 0:HF], in_=dyn_t[:, 0:HF])
        nc.sync.dma_start(out=dyn_dst[:, :, HF:], in_=dyn_t[:, HF:])
        emit_stream_load(2)
        emit_stream_load(3)
        nc.sync.dma_start(out=mfx_dst[:, :, 0:HF], in_=mfx_t[:, 0:HF])
        nc.sync.dma_start(out=mfx_dst[:, :, HF:], in_=mfx_t[:, HF:])
        for i in range(4, len(TILES)):
            emit_stream_load(i)

        # ---- small setup compute (cheap, mostly DVE/gpsimd)
        stask_f = small.tile([B, 1], F32, tag="stask_f")
        nc.vector.tensor_copy(out=stask_f[:, :], in_=stask_i[:, :])
        cs_f = small.tile([B, 1], F32, tag="cs_f")
        nc.vector.tensor_copy(out=cs_f[:, :], in_=cs_i[:, :])

        iota_c = small.tile([P, MC], I32, tag="iota_c")
        nc.gpsimd.iota(iota_c[:, :], pattern=[[1, MC]], channel_multiplier=0)
        iota_cf = small.tile([P, MC], F32, tag="iota_cf")
        nc.vector.tensor_copy(out=iota_cf[:, :], in_=iota_c[:, :])

        # S_b[b, c] = 1.0 if c == couriers[b] else 0.0   (b on partitions 0..63)
        s_onehot = small.tile([B, MC], F32, tag="s_onehot")
        nc.vector.tensor_scalar(
            out=s_onehot[:, :], in0=iota_cf[:B, :], scalar1=cs_f[:, 0:1],
            scalar2=None, op0=Alu.is_equal,
        )
        add_state = small.tile([B, MC], F32, tag="add_state")
        nc.vector.tensor_scalar(
            out=add_state[:, :], in0=s_onehot[:, :], scalar1=stask_f[:, 0:1],
            scalar2=None, op0=Alu.mult,
        )
        add_mask = small.tile([B, MC], F32, tag="add_mask")
        nc.scalar.activation(out=add_mask[:, :], in_=s_onehot[:, :], func=Ln)

        # cs2[r] = couriers[r >> 1]  (row r = 2*b + t_half)
        idx2 = small.tile([P, 1], I32, tag="idx2")
        nc.gpsimd.iota(idx2[:, :], pattern=[[0, 1]], channel_multiplier=1)
        idx2b = small.tile([P, 1], I32, tag="idx2b")
        nc.vector.tensor_scalar(
            out=idx2b[:, :], in0=idx2[:, :], scalar1=1, scalar2=None,
            op0=Alu.arith_shift_right,
        )
        cs2_i = small.tile([P, 1], I32, tag="cs2_i")
        nc.gpsimd.indirect_dma_start(
            out=cs2_i[:, :], out_offset=None, in_=cs[:, :],
            in_offset=IndirectOffsetOnAxis(ap=idx2b[:, 0:1], axis=0),
        )
        cs2_f = small.tile([P, 1], F32, tag="cs2_f")
        nc.vector.tensor_copy(out=cs2_f[:, :], in_=cs2_i[:, :])
        s2_onehot = small.tile([P, MC], F32, tag="s2_onehot")
        nc.vector.tensor_scalar(
            out=s2_onehot[:, :], in0=iota_cf[:, :], scalar1=cs2_f[:, 0:1],
            scalar2=None, op0=Alu.is_equal,
        )
        s2_bcast = s2_onehot[:, :].unsqueeze(1).broadcast_to([P, T2, MC])

        # gather row indices b*128 + courier[b]
        iota_b = small.tile([B, 1], I32, tag="iota_b")
        nc.gpsimd.iota(iota_b[:, :], pattern=[[0, 1]], channel_multiplier=MC)
        idxg = small.tile([B, 1], I32, tag="idxg")
        nc.vector.tensor_tensor(
            out=idxg[:, :], in0=iota_b[:, :], in1=cs_i[:, :], op=Alu.add,
        )

        # ---- stage B + gather work, emitted as chunks interleaved with the
        # mask_fs stream so DVE/ACT/gpsimd stay fed without bursts
        prod = big.tile([P, T2 * MC], F32, tag="prod")
        d_red = small.tile([P, T2], F32, tag="d_red")
        mfc_t = big.tile([P, T2 * MC], F32, tag="mfc_t")
        prod2 = big.tile([P, T2 * MC], F32, tag="prod")
        mf_red = small.tile([P, T2], F32, tag="mf_red")
        mf_val = small.tile([P, T2], F32, tag="mf_val")
        s_t = small.tile([B, K2], F32, tag="s_t")
        sh_t = small.tile([B, E], F32, tag="sh_t")
        t_s = small.tile([B, K], F32, tag="t_s")
        u_s = small.tile([B, K], F32, tag="u_s")
        m_s = small.tile([B, K], F32, tag="m_s")

        def chunk_d_prod():
            nc.vector.tensor_tensor(
                out=prod[:, :].rearrange("p (t c) -> p t c", c=MC),
                in0=dyn_t[:, :].rearrange("p (t c) -> p t c", c=MC),
                in1=s2_bcast,
                op=Alu.mult,
            )

        def chunk_d_red():
            nc.vector.tensor_reduce(
                out=d_red[:, :],
                in_=prod[:, :].rearrange("p (t c) -> p t c", c=MC),
                axis=mybir.AxisListType.X,
                op=Alu.add,
            )
            nc.gpsimd.dma_start(
                out=d_out[:, :, :].rearrange("b (h t) one -> (b h) (t one)", h=2),
                in_=d_red[:, :],
            )

        def chunk_mf_clamp():
            # clamp {0, -inf} -> {0, -1} so the one-hot multiply cannot NaN
            # (gpsimd: 1-input elementwise runs near line rate there, and it
            # keeps DVE free for the mask_fs stream)
            nc.vector.tensor_scalar(
                out=mfc_t[:, :], in0=mfx_t[:, :], scalar1=-1.0, scalar2=None,
                op0=Alu.max,
            )

        def chunk_mf_prod():
            nc.vector.tensor_tensor(
                out=prod2[:, :].rearrange("p (t c) -> p t c", c=MC),
                in0=mfc_t[:, :].rearrange("p (t c) -> p t c", c=MC),
                in1=s2_bcast,
                op=Alu.mult,
            )

        def chunk_mf_red():
            nc.vector.tensor_reduce(
                out=mf_red[:, :],
                in_=prod2[:, :].rearrange("p (t c) -> p t c", c=MC),
                axis=mybir.AxisListType.X,
                op=Alu.add,
            )
            # map {0, -1} back to {0, -inf}: Ln(x + 1)
            nc.scalar.activation(out=mf_val[:, :], in_=mf_red[:, :], func=Ln,
                                 bias=1.0)
            nc.gpsimd.dma_start(
                out=mf_out[:, :, :].rearrange("b (h t) one -> (b h) (t one)", h=2),
                in_=mf_val[:, :],
            )

        def chunk_s_gather():
            nc.gpsimd.indirect_dma_start(
                out=s_t[:, :], out_offset=None,
                in_=static[:, :, :].rearrange("b m k -> (b m) k"),
                in_offset=IndirectOffsetOnAxis(ap=idxg[:, 0:1], axis=0),
            )
            nc.gpsimd.dma_start(out=s_out[:, 0, :], in_=s_t[:, :])

        def chunk_sh_gather():
            nc.gpsimd.indirect_dma_start(
                out=sh_t[:, :], out_offset=None,
                in_=static_h[:, :, :].rearrange("b m k -> (b m) k"),
                in_offset=IndirectOffsetOnAxis(ap=idxg[:, 0:1], axis=0),
            )
            nc.gpsimd.dma_start(out=sh_out[:, 0, :], in_=sh_t[:, :])

        def chunk_mfs():
            # pad predicate on the gathered courier row of static
            nc.vector.tensor_reduce(
                out=t_s[:, :],
                in_=s_t[:, :].rearrange("p (k two) -> p k two", two=2),
                axis=mybir.AxisListType.X,
                op=Alu.max,
                apply_absolute_value=True,
            )
            nc.vector.memset(t_s[:, 0:1], 0.0)
            nc.scalar.activation(out=u_s[:, :], in_=t_s[:, :], func=Sign)
            nc.scalar.activation(out=m_s[:, :], in_=u_s[:, :], func=Ln)
            nc.gpsimd.dma_start(out=mfs_out[:, 0, :], in_=m_s[:, :])

        def chunk_small_outs():
            nc.gpsimd.dma_start(out=dyn_new[:, T, :], in_=add_state[:, :])
            nc.gpsimd.dma_start(out=mf_new[:, T, :], in_=add_mask[:, :])

        chunks = [
            chunk_d_prod, chunk_d_red, chunk_mf_clamp, chunk_mf_prod,
            chunk_mf_red, chunk_s_gather, chunk_sh_gather, chunk_mfs,
            chunk_small_outs,
        ]

        # ---- mask_fs stream compute; stores issue from ACT (wait-free)
        for i in range(len(TILES)):
            st = st_tiles[i]
            b0, nb = TILES[i]
            t_t = stream_mid.tile([P, BPT, K], F32, tag="t_t")
            nc.vector.tensor_reduce(
                out=t_t[:, 0:nb, :],
                in_=st[:, 0:nb, :].rearrange("p b (k two) -> p b k two", two=2),
                axis=mybir.AxisListType.X,
                op=Alu.max,
                apply_absolute_value=True,
            )
            nc.vector.memset(t_t[:, 0:nb, 0:1], 0.0)
            u_t = stream_mid.tile([P, BPT, K], F32, tag="u_t")
            nc.scalar.activation(
                out=u_t[:, 0:nb, :].rearrange("p b k -> p (b k)"),
                in_=t_t[:, 0:nb, :].rearrange("p b k -> p (b k)"),
                func=Sign,
            )
            m_t = stream_out.tile([P, BPT, K], F32, tag="m_t")
            nc.scalar.activation(
                out=m_t[:, 0:nb, :].rearrange("p b k -> p (b k)"),
                in_=u_t[:, 0:nb, :].rearrange("p b k -> p (b k)"),
                func=Ln,
            )
            nc.scalar.dma_start(
                out=mask_fs[b0:b0 + nb, :, :].rearrange("b m k -> m b k"),
                in_=m_t[:, 0:nb, :],
            )
            if chunks:
                chunks.pop(0)()
            if i >= NTILES - 2 and chunks:
                chunks.pop(0)()
        while chunks:
            chunks.pop(0)()


_NC_CACHE = None


def build_nc():
    global _NC_CACHE
    if _NC_CACHE is not None:
        return _NC_CACHE
    nc = bacc.Bacc(trn_type="TRN2")
    with TileContext(nc) as tc:
        _build_body(tc)
    nc.compile()
    _NC_CACHE = nc
    return nc


def kernel(static, static_h, dynamic, mask_f, couriers_selected,
           sensingtask_selected, trace=False):
    from concourse import bass_utils

    nc = build_nc()

    static = np.ascontiguousarray(static, dtype=np.float32)
    static_h = np.ascontiguousarray(static_h, dtype=np.float32)
    dynamic = np.ascontiguousarray(dynamic, dtype=np.float32)
    mask_f = np.ascontiguousarray(mask_f, dtype=np.float32)
    couriers_selected = np.ascontiguousarray(couriers_selected, dtype=np.int32)
    sensingtask_selected = np.ascontiguousarray(sensingtask_selected, dtype=np.int32)

    in_maps = []
    for c in range(NCORES):
        sl = slice(c * B, (c + 1) * B)
        in_maps.append({
            "static": static[sl],
            "static_h": static_h[sl],
            "dynamic": dynamic[sl],
            "mask_f": mask_f[sl],
            "couriers_selected": couriers_selected[sl],
            "sensingtask_selected": sensingtask_selected[sl],
        })

    res = bass_utils.run_bass_kernel_spmd(
        nc, in_maps, core_ids=list(range(NCORES)), trace=trace,
    )
    outs = res.results

    def cat(name):
        return np.concatenate([outs[c][name] for c in range(NCORES)], axis=0)

    result = (
        cat("mask_fs"), cat("s"), cat("sh"), cat("d"), cat("mf"),
        cat("mfs"), cat("dynamic_new"), cat("mask_f_new"),
    )
    if trace:
        return result, res
    return result
